# revision 17
# baseline (speedup 1.0000x reference)
"""Distributed real SHT (spherical harmonic transform) on 8 trn2 NeuronCores.

Pipeline:
  out[b,c,l,m] = sum_k W[m,l,k] * XF[b,c,m,k],   XF = (2*pi/nlon) * rfft(x, lon)[..., :mmax]

Stage A (launch 1, channel-sharded): DFT along longitude as bf16 matmuls.
  Host folds x over lon parity (cos: n'=0..360, sin: n'=1..359), transposes to
  [c, n', k] (contraction n' on partitions), pads n' chunks to 3x128 per parity
  and k to 362, packs cos+sin into one tensor so each channel loads with ONE DMA.
  psum[k_tile, m] += xT[n'chunk, k_tile]^T @ DFTmat[n'chunk, m]
Host exchange: XF[c,k,m] (channel-sharded) -> XFB[m_local, k, (ri,c)] (m-sharded).
Stage B (launch 2, m-sharded, m interleaved mod 8 for triangular balance):
  psum[l_tile, 512] += WT[m][k, l_tile]^T @ XFB[m][k, (ri,c)=512]
  Only l >= m is computed (weights are exactly zero below the diagonal).

bf16 operands keep the PE at 2.4 GHz (fp32r matmuls don't count as HAM activity
and pin the array at 1.2 GHz) and halve DMA bytes; psum accumulation is fp32.
k is padded to 384=3x128 so rhs/weight loads are one rearranged DMA per tile.
"""

import os

import numpy as np

import concourse.bacc as bacc
import concourse.mybir as mybir
from concourse.tile import TileContext
from concourse.bass_utils import run_bass_kernel_spmd

LAST_PERF = {}

NLAT = 361
NLON = 720
MMAX = 361
LMAX = 361
C = 256
NCORES = 8
CPC = C // NCORES  # 32 channels per core
NC_COS = NLON // 2 + 1  # 361 cos columns (n'=0..360)
NC_SIN = NLON // 2 - 1  # 359 sin columns (n'=1..359)
MPC = (MMAX + NCORES - 1) // NCORES  # 46 m's per core (padded)
KPAD = 384  # nlat padded to 3x128 partition chunks
NPAD = 384  # per-parity n' padded to 3x128
MEVEN = 362  # k (stage A moving free dim) padded even

F32 = mybir.dt.float32
BF16 = mybir.dt.bfloat16


def _ptiles(n, p=128):
    out = []
    o = 0
    while o < n:
        out.append((o, min(p, n - o)))
        o += p
    return out


def build_stage_a(cpc=CPC, nlat=NLAT, mmax=MMAX):
    """Inputs: xin [cpc, 2*NPAD, MEVEN] bf16 (cos rows 0:361, sin rows 384:743,
    both zero-padded; last col zero), mats [2*NPAD, MEVEN] bf16 (same row
    packing; DFT matrices with the 2*pi/nlon scale; col dim is m padded even).
    Outputs: xfr/xfi [cpc, KPAD, mmax] bf16 (k rows >= 361 are garbage)."""
    nc = bacc.Bacc("TRN2", target_bir_lowering=False)
    xin = nc.dram_tensor("xin", [cpc, 2 * NPAD, MEVEN], BF16, kind="ExternalInput")
    mats = nc.dram_tensor("mats", [2 * NPAD, MEVEN], BF16, kind="ExternalInput")
    xfr = nc.dram_tensor("xfr", [cpc, KPAD, mmax], BF16, kind="ExternalOutput")
    xfi = nc.dram_tensor("xfi", [cpc, KPAD, mmax], BF16, kind="ExternalOutput")

    nseg = 2 * NPAD // 128  # 6 partition segments: 3 cos + 3 sin
    k_tiles = _ptiles(nlat)  # psum partition tiles over k (128,128,105)
    copy_idx = 0
    with TileContext(nc) as tc:
        with (
            tc.tile_pool(name="mats", bufs=1) as matp,
            tc.tile_pool(name="xinp", bufs=4) as xinp,
            tc.tile_pool(name="outp", bufs=6) as outp,
            tc.tile_pool(name="ps", bufs=8, space="PSUM") as psp,
        ):
            mat_t = matp.tile([128, nseg * MEVEN], BF16, tag="mats")
            nc.sync.dma_start(
                out=mat_t.rearrange("p (s m) -> p s m", s=nseg),
                in_=mats.rearrange("(s p) m -> p s m", p=128),
            )

            for c in range(cpc):
                x_t = xinp.tile([128, nseg * MEVEN], BF16, tag="xin")
                nc.sync.dma_start(
                    out=x_t.rearrange("p (s k) -> p s k", s=nseg),
                    in_=xin[c].rearrange("(s p) k -> p s k", p=128),
                )
                for ri, odram in enumerate((xfr, xfi)):
                    ot = outp.tile([128, len(k_tiles) * mmax], BF16, tag="ot")
                    kp_last = k_tiles[-1][1]
                    if kp_last < 128:
                        base = (kp_last // 32) * 32  # partition offsets must be 32-aligned
                        nc.gpsimd.memset(
                            ot[base:, (len(k_tiles) - 1) * mmax :], 0.0
                        )
                    for kt, (k0, kp) in enumerate(k_tiles):
                        ps = psp.tile([128, mmax + 1], F32, tag="ps")
                        for s in range(3):
                            seg = 3 * ri + s
                            nc.tensor.matmul(
                                ps[:kp, :],
                                x_t[:, seg * MEVEN + k0 : seg * MEVEN + k0 + kp],
                                mat_t[:, seg * MEVEN : seg * MEVEN + mmax + 1],
                                start=(s == 0),
                                stop=(s == 2),
                            )
                        dst = ot[:kp, kt * mmax : (kt + 1) * mmax]
                        if copy_idx % 2 == 0:
                            nc.vector.tensor_copy(out=dst, in_=ps[:kp, :mmax])
                        else:
                            nc.scalar.copy(dst, ps[:kp, :mmax])
                        copy_idx += 1
                    nc.gpsimd.dma_start(
                        out=odram[c].rearrange("(t p) m -> p t m", p=128),
                        in_=ot.rearrange("p (t m) -> p t m", t=len(k_tiles)),
                    )
    nc.compile()
    return nc


def build_stage_b(mpc=MPC, nlat=NLAT, lmax=LMAX, ncores=NCORES):
    """Inputs: xfb [mpc, KPAD, 512] bf16 (k rows >= 361 zero),
    wt [mpc, KPAD, lmax] bf16 (k rows >= 361 zero) -> out [mpc, lmax, 512] f32.
    Index i handles m = ncores*i + core_j; computes l in [ncores*i, lmax)
    uniformly across cores (weights are zero for l < m -> exact zeros)."""
    nc = bacc.Bacc("TRN2", target_bir_lowering=False)
    nric = 2 * C
    xfb = nc.dram_tensor("xfb", [mpc, KPAD, nric], BF16, kind="ExternalInput")
    wt = nc.dram_tensor("wt", [mpc, KPAD, lmax], BF16, kind="ExternalInput")
    out = nc.dram_tensor("out", [mpc, lmax, nric], F32, kind="ExternalOutput")

    nkc = KPAD // 128  # 3 uniform k chunks (tail rows are zero)
    copy_idx = 0
    with TileContext(nc) as tc:
        with (
            tc.tile_pool(name="rhs", bufs=8) as rhsp,
            tc.tile_pool(name="wts", bufs=8) as wtp,
            tc.tile_pool(name="outp", bufs=8) as outp,
            tc.tile_pool(name="ps", bufs=6, space="PSUM") as psp,
        ):
            for i in range(mpc):
                rhs_t = rhsp.tile([128, nkc * nric], BF16, tag="rhs")
                eng_a = nc.sync if i % 2 == 0 else nc.scalar
                eng_b = nc.scalar if i % 2 == 0 else nc.sync
                eng_a.dma_start(
                    out=rhs_t.rearrange("p (t f) -> p t f", t=nkc),
                    in_=xfb[i].rearrange("(t p) f -> p t f", p=128),
                )
                l_lo = ncores * i
                w_t = wtp.tile([128, nkc * lmax], BF16, tag="wt")
                # opposite HWDGE ring from rhs; only the l >= l_lo triangle
                eng_b.dma_start(
                    out=w_t.rearrange("p (t l) -> p t l", t=nkc)[:, :, l_lo:],
                    in_=wt[i, :, l_lo:].rearrange("(t p) l -> p t l", p=128),
                )
                for l0, lp in _ptiles(lmax - l_lo):
                    la = l_lo + l0
                    ps = psp.tile([128, nric], F32, tag="ps")
                    for kc in range(nkc):
                        nc.tensor.matmul(
                            ps[:lp, :],
                            w_t[:, kc * lmax + la : kc * lmax + la + lp],
                            rhs_t[:, kc * nric : (kc + 1) * nric],
                            start=(kc == 0),
                            stop=(kc == nkc - 1),
                        )
                    ot = outp.tile([128, nric], F32, tag="ot")
                    if copy_idx % 2 == 0:
                        nc.vector.tensor_copy(out=ot[:lp, :], in_=ps[:lp, :])
                    else:
                        nc.scalar.copy(ot[:lp, :], ps[:lp, :])
                    copy_idx += 1
                    nc.gpsimd.dma_start(out=out[i, la : la + lp, :], in_=ot[:lp, :])
    nc.compile()
    return nc


def _dft_matrices():
    """cosm[n', m] = s*cos(2 pi m n'/nlon), n'=0..360
    sinm[n', m] = -s*sin(2 pi m n'/nlon), n'=1..359 (imag of rfft = -sum x sin)."""
    s = 2.0 * np.pi / NLON
    m = np.arange(MMAX)
    nc_ = np.arange(NC_COS)
    ns_ = np.arange(1, NLON // 2)
    ang_c = 2.0 * np.pi * ((nc_[:, None] * m[None, :]) % NLON) / NLON
    ang_s = 2.0 * np.pi * ((ns_[:, None] * m[None, :]) % NLON) / NLON
    return (s * np.cos(ang_c)).astype(np.float32), (-s * np.sin(ang_s)).astype(
        np.float32
    )


def fold_x(x):
    """x: (C, nlat, nlon) f32 -> xc (C, nlat, 361), xs (C, nlat, 359)."""
    xc = np.empty((x.shape[0], x.shape[1], NC_COS), dtype=np.float32)
    xc[..., 0] = x[..., 0]
    xc[..., NLON // 2] = x[..., NLON // 2]
    xc[..., 1 : NLON // 2] = x[..., 1 : NLON // 2] + x[..., : NLON // 2 : -1]
    xs = x[..., 1 : NLON // 2] - x[..., : NLON // 2 : -1]
    return xc, np.ascontiguousarray(xs.astype(np.float32))


def pack_stage_a_inputs(x):
    """x: (C, nlat, nlon) f32 -> xin (C, 768, 362) bf16, mats (768, 362) bf16."""
    import ml_dtypes

    bf = ml_dtypes.bfloat16
    xc, xs = fold_x(x)
    xin = np.zeros((x.shape[0], 2 * NPAD, MEVEN), dtype=bf)
    xin[:, :NC_COS, :NLAT] = xc.transpose(0, 2, 1).astype(bf)
    xin[:, NPAD : NPAD + NC_SIN, :NLAT] = xs.transpose(0, 2, 1).astype(bf)
    cosm, sinm = _dft_matrices()
    mats = np.zeros((2 * NPAD, MEVEN), dtype=bf)
    mats[:NC_COS, :MMAX] = cosm.astype(bf)
    mats[NPAD : NPAD + NC_SIN, :MMAX] = sinm.astype(bf)
    return xin, mats


def m_list(j):
    return [NCORES * i + j for i in range(MPC) if NCORES * i + j < MMAX]


def _install_ntff_hook():
    """This image's antenv lacks axon_hooks; synthesize it so bass_utils'
    trace=True path can capture NTFFs via the axon PJRT .so."""
    import sys

    if "antenv.axon_hooks" in sys.modules:
        return
    import types

    mod = types.ModuleType("antenv.axon_hooks")
    state = {"hook": None}
    mod.set_axon_ntff_profile_hook = lambda h: state.__setitem__("hook", h)
    mod.get_axon_ntff_profile_hook = lambda: state["hook"]
    sys.modules["antenv.axon_hooks"] = mod
    try:
        import importlib.util as ilu

        spec = ilu.spec_from_file_location(
            "_trn_boot_hook", "/root/.axon_site/trn_agent_boot/trn_boot.py"
        )
        tb = ilu.module_from_spec(spec)
        spec.loader.exec_module(tb)
        mod.set_axon_ntff_profile_hook(
            tb._ntff_profile_via_ctypes("/opt/axon/libaxon_pjrt.so")
        )
    except Exception:
        pass


def _run(nc, in_maps, label):
    kw = {}
    if os.environ.get("SHT_TRACE"):
        import concourse.bass_utils as bu

        bu.upload_artifacts = lambda tmpdir: tmpdir  # no S3 in this sandbox
        _install_ntff_hook()
        kw = dict(trace=True)
    try:
        res = run_bass_kernel_spmd(nc, in_maps, core_ids=list(range(NCORES)), **kw)
    except Exception:
        if not kw:
            raise
        res = run_bass_kernel_spmd(nc, in_maps, core_ids=list(range(NCORES)))
    LAST_PERF[label] = res.exec_time_ns
    return res


def kernel(x, weights):
    import ml_dtypes

    bf = ml_dtypes.bfloat16
    x = np.asarray(x, dtype=np.float32).reshape(C, NLAT, NLON)
    weights = np.asarray(weights, dtype=np.float32)

    xin, mats = pack_stage_a_inputs(x)
    nc_a = build_stage_a()
    in_maps = [
        {"xin": xin[j * CPC : (j + 1) * CPC], "mats": mats} for j in range(NCORES)
    ]
    res_a = _run(nc_a, in_maps, "stage_a")
    # (C, k, m), drop k padding rows
    xfr = np.concatenate([r["xfr"][:, :NLAT, :] for r in res_a.results], axis=0)
    xfi = np.concatenate([r["xfi"][:, :NLAT, :] for r in res_a.results], axis=0)

    wtf = weights.transpose(0, 2, 1).astype(bf)  # (m, k, l)
    in_maps_b = []
    for j in range(NCORES):
        ml = m_list(j)
        xfb = np.zeros((MPC, KPAD, 2 * C), dtype=bf)
        xfb[: len(ml), :NLAT, :C] = xfr[:, :, ml].transpose(2, 1, 0)
        xfb[: len(ml), :NLAT, C:] = xfi[:, :, ml].transpose(2, 1, 0)
        wtj = np.zeros((MPC, KPAD, LMAX), dtype=bf)
        wtj[: len(ml), :NLAT] = wtf[ml]
        in_maps_b.append({"xfb": xfb, "wt": wtj})
    nc_b = build_stage_b()
    res_b = _run(nc_b, in_maps_b, "stage_b")

    out = np.zeros((1, C, LMAX, MMAX), dtype=np.complex64)
    for j in range(NCORES):
        ml = m_list(j)
        o = np.asarray(res_b.results[j]["out"][: len(ml)], dtype=np.float32)
        out[0][:, :, ml] = (o[:, :, :C] + 1j * o[:, :, C:]).transpose(2, 1, 0)
    return out


# revision 19
# speedup vs baseline: 1.1194x; 1.1194x over previous
"""Distributed real SHT (spherical harmonic transform) on 8 trn2 NeuronCores.

Pipeline:
  out[b,c,l,m] = sum_k W[m,l,k] * XF[b,c,m,k],   XF = (2*pi/nlon) * rfft(x, lon)[..., :mmax]

Stage A (launch 1, channel-sharded): DFT along longitude as bf16 matmuls.
  Host folds x over lon parity (cos: n'=0..360, sin: n'=1..359), transposes to
  [c, n', k] (contraction n' on partitions), pads n' chunks to 3x128 per parity
  and k to 362, packs cos+sin into one tensor so each channel loads with ONE DMA.
  psum[k_tile, m] += xT[n'chunk, k_tile]^T @ DFTmat[n'chunk, m]
Host exchange: XF[c,k,m] (channel-sharded) -> XFB[m_local, k, (ri,c)] (m-sharded).
Stage B (launch 2, m-sharded, m interleaved mod 8 for triangular balance):
  psum[l_tile, 512] += WT[m][k, l_tile]^T @ XFB[m][k, (ri,c)=512]
  Only l >= m is computed (weights are exactly zero below the diagonal).

bf16 operands keep the PE at 2.4 GHz (fp32r matmuls don't count as HAM activity
and pin the array at 1.2 GHz) and halve DMA bytes; psum accumulation is fp32.
k is padded to 384=3x128 so rhs/weight loads are one rearranged DMA per tile.
"""

import os

import numpy as np

import concourse.bacc as bacc
import concourse.mybir as mybir
from concourse.tile import TileContext
from concourse.bass_utils import run_bass_kernel_spmd

LAST_PERF = {}

NLAT = 361
NLON = 720
MMAX = 361
LMAX = 361
C = 256
NCORES = 8
CPC = C // NCORES  # 32 channels per core
NC_COS = NLON // 2 + 1  # 361 cos columns (n'=0..360)
NC_SIN = NLON // 2 - 1  # 359 sin columns (n'=1..359)
MPC = (MMAX + NCORES - 1) // NCORES  # 46 m's per core (padded)
KPAD = 384  # nlat padded to 3x128 partition chunks
NPAD = 384  # per-parity n' padded to 3x128
MEVEN = 362  # k (stage A moving free dim) padded even

F32 = mybir.dt.float32
BF16 = mybir.dt.bfloat16


def _ptiles(n, p=128):
    out = []
    o = 0
    while o < n:
        out.append((o, min(p, n - o)))
        o += p
    return out


def build_stage_a(cpc=CPC, nlat=NLAT, mmax=MMAX):
    """Inputs: xin [cpc, 2*NPAD, MEVEN] bf16 (cos rows 0:361, sin rows 384:743,
    both zero-padded; last col zero), mats [2*NPAD, MEVEN] bf16 (same row
    packing; DFT matrices with the 2*pi/nlon scale; col dim is m padded even).
    Outputs: xfr/xfi [cpc, KPAD, mmax] bf16 (k rows >= 361 are garbage)."""
    nc = bacc.Bacc("TRN2", target_bir_lowering=False)
    xin = nc.dram_tensor("xin", [cpc, 2 * NPAD, MEVEN], BF16, kind="ExternalInput")
    mats = nc.dram_tensor("mats", [2 * NPAD, MEVEN], BF16, kind="ExternalInput")
    xfr = nc.dram_tensor("xfr", [cpc, KPAD, mmax], BF16, kind="ExternalOutput")
    xfi = nc.dram_tensor("xfi", [cpc, KPAD, mmax], BF16, kind="ExternalOutput")

    nseg = 2 * NPAD // 128  # 6 partition segments: 3 cos + 3 sin
    k_tiles = _ptiles(nlat)  # psum partition tiles over k (128,128,105)
    copy_idx = 0
    with TileContext(nc) as tc:
        with (
            tc.tile_pool(name="mats", bufs=1) as matp,
            tc.tile_pool(name="xinp", bufs=3) as xinp,
            tc.tile_pool(name="outp", bufs=4) as outp,
            tc.tile_pool(name="ps", bufs=6, space="PSUM") as psp,
        ):
            mat_t = matp.tile([128, nseg * MEVEN], BF16, tag="mats")
            nc.sync.dma_start(
                out=mat_t.rearrange("p (s m) -> p s m", s=nseg),
                in_=mats.rearrange("(s p) m -> p s m", p=128),
            )

            for c in range(cpc):
                x_t = xinp.tile([128, nseg * MEVEN], BF16, tag="xin")
                nc.sync.dma_start(
                    out=x_t.rearrange("p (s k) -> p s k", s=nseg),
                    in_=xin[c].rearrange("(s p) k -> p s k", p=128),
                )
                for ri, odram in enumerate((xfr, xfi)):
                    ot = outp.tile([128, len(k_tiles) * mmax], BF16, tag="ot")
                    kp_last = k_tiles[-1][1]
                    if kp_last < 128:
                        base = (kp_last // 32) * 32  # partition offsets must be 32-aligned
                        nc.gpsimd.memset(
                            ot[base:, (len(k_tiles) - 1) * mmax :], 0.0
                        )
                    for kt, (k0, kp) in enumerate(k_tiles):
                        ps = psp.tile([128, mmax + 1], F32, tag="ps")
                        for s in range(3):
                            seg = 3 * ri + s
                            nc.tensor.matmul(
                                ps[:kp, :],
                                x_t[:, seg * MEVEN + k0 : seg * MEVEN + k0 + kp],
                                mat_t[:, seg * MEVEN : seg * MEVEN + mmax + 1],
                                start=(s == 0),
                                stop=(s == 2),
                            )
                        dst = ot[:kp, kt * mmax : (kt + 1) * mmax]
                        if copy_idx % 2 == 0:
                            nc.vector.tensor_copy(out=dst, in_=ps[:kp, :mmax])
                        else:
                            nc.scalar.copy(dst, ps[:kp, :mmax])
                        copy_idx += 1
                    nc.gpsimd.dma_start(
                        out=odram[c].rearrange("(t p) m -> p t m", p=128),
                        in_=ot.rearrange("p (t m) -> p t m", t=len(k_tiles)),
                    )
    nc.compile()
    return nc


def build_stage_b(mpc=MPC, nlat=NLAT, lmax=LMAX, ncores=NCORES, nkc_list=None):
    """Inputs: xfb [mpc, KPAD, 512] bf16, wt [mpc, KPAD, lmax] bf16 -> out
    [mpc, lmax, 512] f32. Index i handles m = ncores*i + core_j; computes l in
    [ncores*i, lmax) uniformly across cores (weights are zero for l < m ->
    exact zeros). nkc_list[i] gives the number of 128-row k chunks actually
    contracted for index i (the host packs only the latitude window where
    P_l^m is non-negligible -- it shrinks toward the equator as m grows)."""
    nc = bacc.Bacc("TRN2", target_bir_lowering=False)
    nric = 2 * C
    xfb = nc.dram_tensor("xfb", [mpc, KPAD, nric], BF16, kind="ExternalInput")
    wt = nc.dram_tensor("wt", [mpc, KPAD, lmax], BF16, kind="ExternalInput")
    out = nc.dram_tensor("out", [mpc, lmax, nric], F32, kind="ExternalOutput")

    nkc_max = KPAD // 128
    if nkc_list is None:
        nkc_list = [nkc_max] * mpc
    copy_idx = 0
    with TileContext(nc) as tc:
        with (
            tc.tile_pool(name="rhs", bufs=8) as rhsp,
            tc.tile_pool(name="wts", bufs=8) as wtp,
            tc.tile_pool(name="outp", bufs=8) as outp,
            tc.tile_pool(name="ps", bufs=6, space="PSUM") as psp,
        ):
            for i in range(mpc):
                nkc = nkc_list[i]
                rhs_t = rhsp.tile([128, nkc_max * nric], BF16, tag="rhs")
                eng_a = nc.sync if i % 2 == 0 else nc.scalar
                eng_b = nc.scalar if i % 2 == 0 else nc.sync
                eng_a.dma_start(
                    out=rhs_t.rearrange("p (t f) -> p t f", t=nkc_max)[:, :nkc],
                    in_=xfb[i, : nkc * 128].rearrange("(t p) f -> p t f", p=128),
                )
                l_lo = ncores * i
                w_t = wtp.tile([128, nkc_max * lmax], BF16, tag="wt")
                # opposite HWDGE ring from rhs; only the l >= l_lo triangle
                eng_b.dma_start(
                    out=w_t.rearrange("p (t l) -> p t l", t=nkc_max)[:, :nkc, l_lo:],
                    in_=wt[i, : nkc * 128, l_lo:].rearrange("(t p) l -> p t l", p=128),
                )
                for l0, lp in _ptiles(lmax - l_lo):
                    la = l_lo + l0
                    ps = psp.tile([128, nric], F32, tag="ps")
                    for kc in range(nkc):
                        nc.tensor.matmul(
                            ps[:lp, :],
                            w_t[:, kc * lmax + la : kc * lmax + la + lp],
                            rhs_t[:, kc * nric : (kc + 1) * nric],
                            start=(kc == 0),
                            stop=(kc == nkc - 1),
                        )
                    ot = outp.tile([128, nric], F32, tag="ot")
                    nc.vector.tensor_copy(out=ot[:lp, :], in_=ps[:lp, :])
                    nc.gpsimd.dma_start(out=out[i, la : la + lp, :], in_=ot[:lp, :])
    nc.compile()
    return nc


def _dft_matrices():
    """cosm[n', m] = s*cos(2 pi m n'/nlon), n'=0..360
    sinm[n', m] = -s*sin(2 pi m n'/nlon), n'=1..359 (imag of rfft = -sum x sin)."""
    s = 2.0 * np.pi / NLON
    m = np.arange(MMAX)
    nc_ = np.arange(NC_COS)
    ns_ = np.arange(1, NLON // 2)
    ang_c = 2.0 * np.pi * ((nc_[:, None] * m[None, :]) % NLON) / NLON
    ang_s = 2.0 * np.pi * ((ns_[:, None] * m[None, :]) % NLON) / NLON
    return (s * np.cos(ang_c)).astype(np.float32), (-s * np.sin(ang_s)).astype(
        np.float32
    )


def fold_x(x):
    """x: (C, nlat, nlon) f32 -> xc (C, nlat, 361), xs (C, nlat, 359)."""
    xc = np.empty((x.shape[0], x.shape[1], NC_COS), dtype=np.float32)
    xc[..., 0] = x[..., 0]
    xc[..., NLON // 2] = x[..., NLON // 2]
    xc[..., 1 : NLON // 2] = x[..., 1 : NLON // 2] + x[..., : NLON // 2 : -1]
    xs = x[..., 1 : NLON // 2] - x[..., : NLON // 2 : -1]
    return xc, np.ascontiguousarray(xs.astype(np.float32))


def pack_stage_a_inputs(x):
    """x: (C, nlat, nlon) f32 -> xin (C, 768, 362) bf16, mats (768, 362) bf16."""
    import ml_dtypes

    bf = ml_dtypes.bfloat16
    xc, xs = fold_x(x)
    xin = np.zeros((x.shape[0], 2 * NPAD, MEVEN), dtype=bf)
    xin[:, :NC_COS, :NLAT] = xc.transpose(0, 2, 1).astype(bf)
    xin[:, NPAD : NPAD + NC_SIN, :NLAT] = xs.transpose(0, 2, 1).astype(bf)
    cosm, sinm = _dft_matrices()
    mats = np.zeros((2 * NPAD, MEVEN), dtype=bf)
    mats[:NC_COS, :MMAX] = cosm.astype(bf)
    mats[NPAD : NPAD + NC_SIN, :MMAX] = sinm.astype(bf)
    return xin, mats


def m_list(j):
    return [NCORES * i + j for i in range(MPC) if NCORES * i + j < MMAX]


def _install_ntff_hook():
    """This image's antenv lacks axon_hooks; synthesize it so bass_utils'
    trace=True path can capture NTFFs via the axon PJRT .so."""
    import sys

    if "antenv.axon_hooks" in sys.modules:
        return
    import types

    mod = types.ModuleType("antenv.axon_hooks")
    state = {"hook": None}
    mod.set_axon_ntff_profile_hook = lambda h: state.__setitem__("hook", h)
    mod.get_axon_ntff_profile_hook = lambda: state["hook"]
    sys.modules["antenv.axon_hooks"] = mod
    try:
        import importlib.util as ilu

        spec = ilu.spec_from_file_location(
            "_trn_boot_hook", "/root/.axon_site/trn_agent_boot/trn_boot.py"
        )
        tb = ilu.module_from_spec(spec)
        spec.loader.exec_module(tb)
        mod.set_axon_ntff_profile_hook(
            tb._ntff_profile_via_ctypes("/opt/axon/libaxon_pjrt.so")
        )
    except Exception:
        pass


def _run(nc, in_maps, label):
    kw = {}
    if os.environ.get("SHT_TRACE"):
        import concourse.bass_utils as bu

        bu.upload_artifacts = lambda tmpdir: tmpdir  # no S3 in this sandbox
        _install_ntff_hook()
        kw = dict(trace=True)
    try:
        res = run_bass_kernel_spmd(nc, in_maps, core_ids=list(range(NCORES)), **kw)
    except Exception:
        if not kw:
            raise
        res = run_bass_kernel_spmd(nc, in_maps, core_ids=list(range(NCORES)))
    LAST_PERF[label] = res.exec_time_ns
    return res


def kernel(x, weights):
    import ml_dtypes

    bf = ml_dtypes.bfloat16
    x = np.asarray(x, dtype=np.float32).reshape(C, NLAT, NLON)
    weights = np.asarray(weights, dtype=np.float32)

    xin, mats = pack_stage_a_inputs(x)
    nc_a = build_stage_a()
    in_maps = [
        {"xin": xin[j * CPC : (j + 1) * CPC], "mats": mats} for j in range(NCORES)
    ]
    res_a = _run(nc_a, in_maps, "stage_a")
    # (C, k, m), drop k padding rows
    xfr = np.concatenate([r["xfr"][:, :NLAT, :] for r in res_a.results], axis=0)
    xfi = np.concatenate([r["xfi"][:, :NLAT, :] for r in res_a.results], axis=0)

    wtf = weights.transpose(0, 2, 1).astype(bf)  # (m, k, l)
    # per-index latitude windows: union of |W| support over the 8 cores' m's
    wabs = np.abs(weights).max(axis=1)  # (m, k)
    thr = 1e-7 * wabs.max()
    windows = []
    for i in range(MPC):
        ms = [NCORES * i + j for j in range(NCORES) if NCORES * i + j < MMAX]
        nz = np.nonzero(wabs[ms].max(axis=0) > thr)[0]
        klo, khi = (int(nz[0]), int(nz[-1]) + 1) if len(nz) else (0, NLAT)
        span = min(-(-max(khi - klo, 1) // 128) * 128, KPAD)
        klo = max(0, min(klo, NLAT - span)) if span < NLAT else 0
        windows.append((klo, span))
    nkc_list = [span // 128 for _, span in windows]
    in_maps_b = []
    for j in range(NCORES):
        ml = m_list(j)
        xfb = np.zeros((MPC, KPAD, 2 * C), dtype=bf)
        wtj = np.zeros((MPC, KPAD, LMAX), dtype=bf)
        for i in range(MPC):
            m = NCORES * i + j
            if m >= MMAX:
                continue
            klo, span = windows[i]
            khi = min(klo + span, NLAT)
            n = khi - klo
            xfb[i, :n, :C] = xfr[:, klo:khi, m].T
            xfb[i, :n, C:] = xfi[:, klo:khi, m].T
            wtj[i, :n] = wtf[m, klo:khi]
        in_maps_b.append({"xfb": xfb, "wt": wtj})
    nc_b = build_stage_b(nkc_list=nkc_list)
    res_b = _run(nc_b, in_maps_b, "stage_b")

    out = np.zeros((1, C, LMAX, MMAX), dtype=np.complex64)
    for j in range(NCORES):
        ml = m_list(j)
        o = np.asarray(res_b.results[j]["out"][: len(ml)], dtype=np.float32)
        out[0][:, :, ml] = (o[:, :, :C] + 1j * o[:, :, C:]).transpose(2, 1, 0)
    return out
